# revision 61
# baseline (speedup 1.0000x reference)
"""MCCDecoderAttention Trainium2 kernel (8 NeuronCores) — v3 (all-bf16).

Sharding: core = b*4 + g  (b in {0,1} batch, g in {0..3} head-group).
Each core computes attention for 3 heads of one batch plus its partial
contribution to the output projection; the host sums the 4 partials per
batch and adds b_proj.

Numerics: everything is bf16 (fp8 fails here: max logits reach ~9.5 so
exp overflows fp8e4m3, and the softmax is concentrated enough that fp8
quantization of A or V alone costs 3-4e-2 relative error).  Measured
end-to-end error of this scheme is ~7e-3 against the f32 reference.

Schedule (learned from TimelineSim iteration):
  * The PE executes its queue in order, so emission order is the
    schedule: S^T matmuls run one pair ahead of the exp stream, each
    finished chunk's projection is interleaved into the next chunk's
    pair stream, and chunk-3 QKV work is injected after the stream
    starts (its DMA lands last).
  * exp runs on ACT; PSUM-touching copies on DVE (GPSIMD cannot access
    PSUM); 1/L row broadcast via gpsimd partition_broadcast (source must
    sit at partition 0 of its tile - the HW ignores AP offsets there).
  * Output projection contracts K=192 as 128+64 via a stacked ao tile
    (cross-partition elementwise *output* is HW-legal; cross-partition
    inputs are not, hence the aligned-copy in the diag path).
  * Throwaway warm-up matmuls run while the input DMA is in flight so
    the PE clock (HAM) is at full rate when the real QKV work starts.

Decoder mask (last `u` keys masked except the diagonal) is handled by
looping keys over [0, N-u) plus an elementwise diagonal correction for
queries in the unseen range (per-head V^T of the unseen tokens: vtu).
"""

import functools
import os
import sys

for _p in ("/opt/trn_rl_repo", "/root/.axon_site/_ro/trn_rl_repo"):
    if os.path.isdir(_p) and _p not in sys.path:
        sys.path.insert(0, _p)

import numpy as np
import ml_dtypes

import concourse.bacc as bacc
import concourse.tile as tile
from concourse import mybir

N, C, D = 2048, 768, 64
NH = 3            # heads per core
F32 = mybir.dt.float32
BF16 = mybir.dt.bfloat16
EXP = mybir.ActivationFunctionType.Exp

_last_results = None  # BassKernelResults of the most recent run (for test.py)


@functools.lru_cache(maxsize=4)
def _build(u: int):
    nc = bacc.Bacc(None, target_bir_lowering=False)
    xd = nc.dram_tensor("x16", [128, 6, N], BF16, kind="ExternalInput")
    wqkd = nc.dram_tensor("wqk16", [128, 3, 6, 128], BF16,
                          kind="ExternalInput")
    wvd = nc.dram_tensor("wv16", [128, 6, 192], BF16, kind="ExternalInput")
    wpAd = nc.dram_tensor("wpA", [128, C], BF16, kind="ExternalInput")
    wpBd = nc.dram_tensor("wpB", [64, C], BF16, kind="ExternalInput")
    yd = nc.dram_tensor("yT16", [C, N], BF16, kind="ExternalOutput")

    kfull = N - u
    T = (kfull + 127) // 128       # 128-key tiles covering the visible keys
    PAIRS = (T + 1) // 2           # pairs of key tiles (one st/exp per pair)
    rem = kfull - (T - 1) * 128    # valid keys in the last 128-tile (1..128)
    NQ = N // 512                  # query chunks
    # diag-corrected chunk early, cheap chunk last (short tail)
    qorder = [0] + list(range(NQ - 1, 0, -1)) if u else list(range(NQ))

    with nc.allow_low_precision(reason="bf16 staging"), \
         tile.TileContext(nc) as tc:
        with tc.tile_pool(name="persist", bufs=1) as P:
            x16 = P.tile([128, 6, N], BF16)
            wqk = P.tile([128, 3, 6, 128], BF16)
            wv = P.tile([128, 6, 192], BF16)
            wpA = P.tile([128, C], BF16)
            wpB = P.tile([64, C], BF16)
            # q/k tiles; per-head q,k share a partition base (matmul and
            # DVE 2-input ops require equal operand bases)
            qA = P.tile([128, N], BF16)   # q_h0 (rows 0:64) | q_h1 (64:128)
            kA = P.tile([128, N], BF16)   # k_h0 | k_h1
            qB = P.tile([64, N], BF16)    # q_h2
            kB = P.tile([64, N], BF16)    # k_h2
            # V token-major: [part=token%128, tile, head, 66] (64=V, col 64=1)
            vsb = P.tile([128, 16, NH, 66], BF16)
            vtu = [P.tile([64, max(u, 1)], BF16, name=f"vtu{_h}", tag=f"vtu{_h}")
                   for _h in range(NH)] if u else []
            aoA = P.tile([128, N], BF16)   # heads 0 (rows 0:64), 1 (64:128)
            aoB = P.tile([64, N], BF16)    # head 2
            onesf = P.tile([128, 80], F32)
            ones65 = P.tile([128, 65], BF16)  # diag-reduce lhsT (65 out rows)
            ones65b = ones65[64:128, :]

            # DMA order = first-exp critical path; descriptor gen serializes.
            nc.sync.dma_start(x16[:, :, 0:256], xd[:, :, 0:256])
            nc.sync.dma_start(wqk[:, 0, :, :], wqkd[:, 0, :, :])
            nc.sync.dma_start(wqk[:, 1:3, :, :], wqkd[:, 1:3, :, :])
            nc.sync.dma_start(x16[:, :, 256:512], xd[:, :, 256:512])
            nc.sync.dma_start(x16[:, :, 512:1024], xd[:, :, 512:1024])
            nc.sync.dma_start(wv[:], wvd[:])
            nc.sync.dma_start(x16[:, :, 1024:1536], xd[:, :, 1024:1536])
            nc.sync.dma_start(x16[:, :, 1536:2048], xd[:, :, 1536:2048])
            nc.sync.dma_start(wpA[:], wpAd[:])
            nc.sync.dma_start(wpB[:], wpBd[:])

            nc.vector.memset(onesf[:], 1.0)
            nc.vector.tensor_copy(ones65[:], onesf[:, 0:65])
            nc.vector.tensor_copy(
                vsb[:, :, :, 64:65],
                onesf[:, 0:16 * NH].rearrange("p (a b x) -> p a b x", b=NH,
                                              x=1))

            # head -> (q tile, k tile, row base)
            heads = [(qA, kA, 0), (qA, kA, 64), (qB, kB, 0)]

            with tc.tile_pool(name="ps", bufs=1, space="PSUM") as PS, \
                 tc.tile_pool(name="a16p", bufs=3) as ap, \
                 tc.tile_pool(name="scr", bufs=3) as sc, \
                 tc.tile_pool(name="ot", bufs=4) as ot:

                def psA(name):   # [128, 1024] f32 — S^T tiles (2 banks x 2)
                    return PS.tile([128, 1024], F32, name=name, tag="A", bufs=2)

                def psB(name):   # [128, 512] f32 — qkv/v/vtu/proj (1 bank x 2)
                    return PS.tile([128, 512], F32, name=name, tag="B", bufs=2)

                def psC(name):   # [65, 512] f32 — A@V accum + diag (1 bank x 2)
                    return PS.tile([65, 512], F32, name=name, tag="C", bufs=2)

                # warm-up: the PE clock ramps after ~3us of continuous
                # activity (HAM); run throwaway matmuls while the input DMA
                # is still in flight so the real QKV work starts at full rate
                wps = PS.tile([128, 512], F32, name="warm", tag="B", bufs=2)
                for _w in range(34):
                    nc.tensor.matmul(wps[0:65, 0:64], ones65[0:64, :],
                                     ones65[0:64, 0:64],
                                     start=True, stop=True,
                                     skip_group_check=True)

                # ---- phase 1: projections ----
                def emit_qk(c0, c1, early=False):
                    sl = slice(c0, c1)
                    w = c1 - c0
                    # 3 psum groups; the third holds q_h2|k_h2 stacked and
                    # is split into base-0 tiles by two copies (cross-
                    # partition copy *inputs* are HW-legal for one operand)
                    for gi in range(3):
                        ps = psB("qkps")
                        for t in range(6):
                            nc.tensor.matmul(
                                ps[:, 0:w], wqk[:, gi, t, :], x16[:, t, sl],
                                start=(t == 0), stop=(t == 5))
                        if gi < 2:
                            dst = (qA, kA)[gi]
                            if early and gi == 1:
                                # ACT is idle before the first exp
                                nc.scalar.copy(dst[:, sl], ps[:, 0:w])
                            else:
                                nc.vector.tensor_copy(dst[:, sl], ps[:, 0:w])
                        else:
                            nc.vector.tensor_copy(qB[:, sl], ps[0:64, 0:w])
                            eng = nc.scalar if early else nc.vector
                            eng_copy = (eng.copy if early
                                        else nc.vector.tensor_copy)
                            eng_copy(kB[:, sl], ps[64:128, 0:w])

                def emit_v(ch):
                    # V (token-major) for the visible token tiles of a chunk
                    nt0, nt1 = ch * 4, min((ch + 1) * 4, T)
                    for nt in range(nt0, nt1):
                        ps = psB("vps")
                        tsl = slice(nt * 128, nt * 128 + 128)
                        for t in range(6):
                            nc.tensor.matmul(
                                ps[:, 0:192], x16[:, t, tsl], wv[:, t, :],
                                start=(t == 0), stop=(t == 5))
                        nc.vector.tensor_copy(
                            vsb[:, nt, :, 0:64],
                            ps[:, 0:192].rearrange("p (h x) -> p h x", x=64))

                def emit_vtu():
                    # unseen V, feature-major per head (for the diag fixup)
                    for h in range(NH):
                        for uc in range(0, u, 512):
                            w = min(512, u - uc)
                            ps = psB("vtups")
                            for t in range(6):
                                nc.tensor.matmul(
                                    ps[0:64, 0:w],
                                    wv[:, t, h * 64:(h + 1) * 64],
                                    x16[:, t, kfull + uc:kfull + uc + w],
                                    start=(t == 0), stop=(t == 5))
                            nc.vector.tensor_copy(vtu[h][:, uc:uc + w],
                                                  ps[0:64, 0:w])

                def emit_st(Q, h, p):
                    qt, kt, bh = heads[h]
                    full = (2 * p + 2 <= T)
                    nw = 2 if full else 1
                    st = psA("stps")
                    a16 = ap.tile([128, 1024], BF16, tag="a16", name="a16")
                    for i in range(nw):
                        ksl = slice((2 * p + i) * 128, (2 * p + i) * 128 + 128)
                        nc.tensor.matmul(
                            st[:, i * 512:(i + 1) * 512], kt[bh:bh + 64, ksl],
                            qt[bh:bh + 64, Q * 512:Q * 512 + 512],
                            start=True, stop=True)
                    if 2 * p + nw == T and rem < 128:
                        # mask scores of keys >= kfull in the last tile
                        nc.vector.memset(
                            st[rem:128, (nw - 1) * 512:nw * 512], -1e30)
                    return st, a16, nw

                def emit_proj(Q, co, tail_dma=False):
                    qs = Q * 512
                    pj = psB("pjps")
                    nc.tensor.matmul(pj[:], wpA[:, co * 128:(co + 1) * 128],
                                     aoA[:, qs:qs + 512],
                                     start=True, stop=False)
                    nc.tensor.matmul(pj[:], wpB[:, co * 128:(co + 1) * 128],
                                     aoB[:, qs:qs + 512],
                                     start=False, stop=True)
                    o = ot.tile([128, 512], BF16, tag="o", name="o")
                    if tail_dma and co % 2 == 0:
                        nc.scalar.copy(o[:], pj[:])
                    else:
                        nc.vector.tensor_copy(o[:], pj[:])
                    nc.sync.dma_start(yd[co * 128:(co + 1) * 128, qs:qs + 512],
                                      o[:])

                # ---- phase 2+3: pipelined attention + projection ----
                # two (Q, h) blocks run interleaved: while one stream's A@V
                # waits on its exp, the PE executes the other stream's S^T
                # (the in-order PE queue would otherwise idle ~200ns/pair)
                blocks = [(Q, h) for Q in qorder for h in range(NH)]
                tasks = []
                for b0 in range(0, len(blocks), 2):
                    grp = blocks[b0:b0 + 2]
                    for p in range(PAIRS):
                        for Q, h in grp:
                            tasks.append((Q, h, p))
                sts = {}
                # chunk 0 first (two 256-col halves: shortest path to the
                # first S^T pairs), with the leading S^T work interleaved
                emit_qk(0, 256, early=True)
                emit_qk(256, 512, early=True)
                sts[tasks[0]] = emit_st(*tasks[0])
                sts[tasks[1]] = emit_st(*tasks[1])
                emit_qk(512, 1024)
                emit_v(0)
                emit_qk(1024, 1536)
                emit_v(1)
                emit_v(2)

                proj_q = []      # projection tiles ready to interleave
                esbs = {}
                avs = {}
                for idx, (Q, h, p) in enumerate(tasks):
                    qs, qe = Q * 512, Q * 512 + 512
                    us = max(qs, kfull)
                    qt, kt, bh = heads[h]
                    if p == 0:
                        if us < qe and u:
                            # diagonal scores of the unseen queries
                            off = us - qs
                            prod = sc.tile([128, 512], BF16, tag="prod",
                                           name="prod")
                            nc.vector.tensor_mul(prod[bh:bh + 64, off:512],
                                                 qt[bh:bh + 64, us:qe],
                                                 kt[bh:bh + 64, us:qe])
                            dg = psB("dgps")
                            nc.tensor.matmul(dg[0:65, off:512],
                                             ones65[bh:bh + 64, :],
                                             prod[bh:bh + 64, off:512],
                                             start=True, stop=True)
                            esb = sc.tile([65, 512], BF16, tag="esb",
                                          name="esb")
                            nc.scalar.activation(esb[:, off:512],
                                                 dg[0:65, off:512], EXP,
                                                 scale=0.125)
                            esbs[(Q, h)] = esb
                        avs[(Q, h)] = psC("avps")
                    # stay one S^T pair ahead of the exp stream
                    if idx + 1 < len(tasks) and tasks[idx + 1] not in sts:
                        sts[tasks[idx + 1]] = emit_st(*tasks[idx + 1])
                    if idx == 4:
                        emit_qk(1536, 2048)
                        emit_vtu()
                    st, a16, nw = sts.pop((Q, h, p))
                    nc.scalar.activation(
                        a16[:, 0:nw * 512], st[:, 0:nw * 512], EXP,
                        scale=0.125)
                    if proj_q and (idx % 3 == 0 or len(proj_q) > 6):
                        # independent PE work placed before the exp-gated
                        # A@V matmuls; rate-limited so the S^T stream is
                        # never crowded out, but fast enough to drain all
                        # non-final chunks before the tail
                        emit_proj(*proj_q.pop(0))
                    av = avs[(Q, h)]
                    for i in range(nw):
                        nc.tensor.matmul(
                            av[:], vsb[:, 2 * p + i, h, 0:65],
                            a16[:, i * 512:(i + 1) * 512],
                            start=(p == 0 and i == 0),
                            stop=(p == PAIRS - 1 and i == nw - 1),
                            skip_group_check=True)
                    if p < PAIRS - 1:
                        continue
                    avs.pop((Q, h))
                    # ---- softmax normalization: av row 64 is the sum L ----
                    # rc at partition 0: partition_broadcast ignores AP
                    # partition offsets on HW
                    rc = sc.tile([1, 512], F32, tag="rc", name="rc")
                    blc = sc.tile([64, 512], F32, tag="blc", name="blc")
                    dsts = ((aoA, 0), (aoA, 64), (aoB, 0))
                    dt_, db = dsts[h]
                    if us < qe and u:
                        off = us - qs
                        esb = esbs.pop((Q, h))
                        lnew = sc.tile([1, 512], F32, tag="lnew", name="lnew")
                        if off:
                            nc.vector.tensor_copy(lnew[0:1, 0:off],
                                                  av[64:65, 0:off])
                        nc.vector.tensor_add(lnew[0:1, off:512],
                                             av[64:65, off:512],
                                             esb[64:65, off:512])
                        nc.vector.reciprocal(rc[0:1, :], lnew[0:1, :])
                        nc.gpsimd.partition_broadcast(blc[:], rc[0:1, :])
                        if off:
                            nc.vector.tensor_mul(dt_[db:db + 64, qs:us],
                                                 av[0:64, 0:off],
                                                 blc[:, 0:off])
                        t1 = sc.tile([64, 512], BF16, tag="t1", name="t1")
                        t2 = sc.tile([64, 512], F32, tag="t2", name="t2")
                        nc.vector.tensor_mul(t1[:, off:512],
                                             vtu[h][:, us - kfull:qe - kfull],
                                             esb[0:64, off:512])
                        nc.vector.tensor_add(t2[:, off:512],
                                             av[0:64, off:512], t1[:, off:512])
                        nc.vector.tensor_mul(dt_[db:db + 64, us:qe],
                                             t2[:, off:512], blc[:, off:512])
                    elif idx >= len(tasks) - 2:
                        # tail: halve the serial rec->bcast->mul chain
                        nc.vector.reciprocal(rc[0:1, :], av[64:65, :])
                        nc.gpsimd.partition_broadcast(blc[:, 0:256],
                                                      rc[0:1, 0:256])
                        nc.vector.tensor_mul(dt_[db:db + 64, qs:qs + 256],
                                             av[0:64, 0:256], blc[:, 0:256])
                        nc.gpsimd.partition_broadcast(blc[:, 256:512],
                                                      rc[0:1, 256:512])
                        nc.vector.tensor_mul(dt_[db:db + 64, qs + 256:qe],
                                             av[0:64, 256:512],
                                             blc[:, 256:512])
                    else:
                        nc.vector.reciprocal(rc[0:1, :], av[64:65, :])
                        nc.gpsimd.partition_broadcast(blc[:], rc[0:1, :])
                        nc.vector.tensor_mul(dt_[db:db + 64, qs:qe],
                                             av[0:64, :], blc[:])
                    if h == NH - 1:
                        proj_q.extend((Q, co) for co in range(6))
                # drain any straggler tiles of non-final chunks first
                while len(proj_q) > 6:
                    emit_proj(*proj_q.pop(0))
                assert len({q for q, _ in proj_q}) <= 1
                # final chunk's projections: co-pairs share one A-tag
                # psum tile -> one copy + one DMA per pair (fewer serialized
                # descriptor-gens on the tail critical path)
                qs = proj_q[0][0] * 512 if proj_q else 0
                for cp in range(3):
                    pj = psA("pjtail")
                    # wpA half first for both blocks: it only needs aoA
                    # (ready one norm earlier than aoB), so the PE works
                    # through it while the final norm chain completes
                    for ci in range(2):
                        co = 2 * cp + ci
                        csl = slice(ci * 512, (ci + 1) * 512)
                        nc.tensor.matmul(pj[:, csl],
                                         wpA[:, co * 128:(co + 1) * 128],
                                         aoA[:, qs:qs + 512],
                                         start=True, stop=False,
                                         skip_group_check=True)
                    for ci in range(2):
                        co = 2 * cp + ci
                        csl = slice(ci * 512, (ci + 1) * 512)
                        nc.tensor.matmul(pj[:, csl],
                                         wpB[:, co * 128:(co + 1) * 128],
                                         aoB[:, qs:qs + 512],
                                         start=False, stop=True,
                                         skip_group_check=True)
                    o = ot.tile([128, 1024], BF16, tag="o2", name="o2", bufs=3)
                    nc.scalar.copy(o[:, 0:512], pj[:, 0:512])
                    nc.vector.tensor_copy(o[:, 512:1024], pj[:, 512:1024])
                    nc.sync.dma_start(
                        yd[cp * 256:(cp + 1) * 256, qs:qs + 512].rearrange(
                            "(i p) c -> p i c", i=2),
                        o[:].rearrange("p (i c) -> p i c", i=2))

    nc.compile()
    return nc


def _fold(m):
    """[768, F] -> [128, 6, F] bf16: partition p of k-tile t = feature
    128*t + p."""
    F = m.shape[1]
    return np.ascontiguousarray(
        m.reshape(6, 128, F).transpose(1, 0, 2)).astype(ml_dtypes.bfloat16)


def kernel(**inputs):
    global _last_results
    from concourse.bass_utils import run_bass_kernel_spmd

    x = np.asarray(inputs["x"], np.float32)
    w_qkv = np.asarray(inputs["w_qkv"], np.float32)
    w_proj = np.asarray(inputs["w_proj"], np.float32)
    b_proj = np.asarray(inputs["b_proj"], np.float32)
    u = int(np.asarray(inputs["unseen_size"]))
    B = x.shape[0]

    nc = _build(u)

    wT = np.ascontiguousarray(w_qkv.T)         # [768, 2304]
    wpT_full = np.ascontiguousarray(w_proj.T)  # [768, 768] (ci, co)
    x16b = [_fold(np.ascontiguousarray(x[b].T)) for b in range(B)]

    in_maps = []
    for core in range(8):
        b, g = divmod(core, 4)
        hs = [3 * g, 3 * g + 1, 3 * g + 2]
        qcols = [h * D + i for h in hs[:2] for i in range(D)]
        kcols = [C + h * D + i for h in hs[:2] for i in range(D)]
        q2 = [hs[2] * D + i for i in range(D)]
        k2 = [C + hs[2] * D + i for i in range(D)]
        vcols = [2 * C + h * D + i for h in hs for i in range(D)]
        wqk16 = np.ascontiguousarray(
            _fold(wT[:, qcols + kcols + q2 + k2])
            .reshape(128, 6, 3, 128).transpose(0, 2, 1, 3))
        wv16 = _fold(wT[:, vcols])
        ci = [h * D + i for h in hs for i in range(D)]
        wpT = np.ascontiguousarray(wpT_full[ci, :])
        in_maps.append({
            "x16": x16b[b], "wqk16": wqk16, "wv16": wv16,
            "wpA": wpT[0:128].astype(ml_dtypes.bfloat16),
            "wpB": wpT[128:192].astype(ml_dtypes.bfloat16),
        })

    trace = bool(int(os.environ.get("KERNEL_TRACE", "0")))
    res = run_bass_kernel_spmd(nc, in_maps, core_ids=list(range(8)), trace=trace)
    _last_results = res

    y = np.zeros((B, N, C), np.float32)
    for core in range(8):
        b = core // 4
        y[b] += np.asarray(res.results[core]["yT16"]).astype(np.float32).T
    y += b_proj
    return y


# revision 63
# speedup vs baseline: 1.0017x; 1.0017x over previous
"""MCCDecoderAttention Trainium2 kernel (8 NeuronCores) — v3 (all-bf16).

Sharding: core = b*4 + g  (b in {0,1} batch, g in {0..3} head-group).
Each core computes attention for 3 heads of one batch plus its partial
contribution to the output projection; the host sums the 4 partials per
batch and adds b_proj.

Numerics: everything is bf16 (fp8 fails here: max logits reach ~9.5 so
exp overflows fp8e4m3, and the softmax is concentrated enough that fp8
quantization of A or V alone costs 3-4e-2 relative error).  Measured
end-to-end error of this scheme is ~7e-3 against the f32 reference.

Schedule (learned from TimelineSim iteration):
  * The PE executes its queue in order, so emission order is the
    schedule: S^T matmuls run one pair ahead of the exp stream, each
    finished chunk's projection is interleaved into the next chunk's
    pair stream, and chunk-3 QKV work is injected after the stream
    starts (its DMA lands last).
  * exp runs on ACT; PSUM-touching copies on DVE (GPSIMD cannot access
    PSUM); 1/L row broadcast via gpsimd partition_broadcast (source must
    sit at partition 0 of its tile - the HW ignores AP offsets there).
  * Output projection contracts K=192 as 128+64 via a stacked ao tile
    (cross-partition elementwise *output* is HW-legal; cross-partition
    inputs are not, hence the aligned-copy in the diag path).
  * Throwaway warm-up matmuls run while the input DMA is in flight so
    the PE clock (HAM) is at full rate when the real QKV work starts.

Decoder mask (last `u` keys masked except the diagonal) is handled by
looping keys over [0, N-u) plus an elementwise diagonal correction for
queries in the unseen range (per-head V^T of the unseen tokens: vtu).
"""

import functools
import os
import sys

for _p in ("/opt/trn_rl_repo", "/root/.axon_site/_ro/trn_rl_repo"):
    if os.path.isdir(_p) and _p not in sys.path:
        sys.path.insert(0, _p)

import numpy as np
import ml_dtypes

import concourse.bacc as bacc
import concourse.tile as tile
from concourse import mybir

N, C, D = 2048, 768, 64
NH = 3            # heads per core
F32 = mybir.dt.float32
BF16 = mybir.dt.bfloat16
EXP = mybir.ActivationFunctionType.Exp

_last_results = None  # BassKernelResults of the most recent run (for test.py)


@functools.lru_cache(maxsize=4)
def _build(u: int):
    nc = bacc.Bacc(None, target_bir_lowering=False)
    xd = nc.dram_tensor("x16", [128, 6, N], BF16, kind="ExternalInput")
    wqkd = nc.dram_tensor("wqk16", [128, 3, 6, 128], BF16,
                          kind="ExternalInput")
    wvd = nc.dram_tensor("wv16", [128, 6, 192], BF16, kind="ExternalInput")
    wpAd = nc.dram_tensor("wpA", [128, C], BF16, kind="ExternalInput")
    wpBd = nc.dram_tensor("wpB", [64, C], BF16, kind="ExternalInput")
    yd = nc.dram_tensor("yT16", [C, N], BF16, kind="ExternalOutput")

    kfull = N - u
    T = (kfull + 127) // 128       # 128-key tiles covering the visible keys
    PAIRS = (T + 1) // 2           # pairs of key tiles (one st/exp per pair)
    rem = kfull - (T - 1) * 128    # valid keys in the last 128-tile (1..128)
    NQ = N // 512                  # query chunks
    # diag-corrected chunk early, cheap chunk last (short tail)
    qorder = [0] + list(range(NQ - 1, 0, -1)) if u else list(range(NQ))

    with nc.allow_low_precision(reason="bf16 staging"), \
         tile.TileContext(nc) as tc:
        with tc.tile_pool(name="persist", bufs=1) as P:
            x16 = P.tile([128, 6, N], BF16)
            wqk = P.tile([128, 3, 6, 128], BF16)
            wv = P.tile([128, 6, 192], BF16)
            wpA = P.tile([128, C], BF16)
            wpB = P.tile([64, C], BF16)
            # q/k tiles; per-head q,k share a partition base (matmul and
            # DVE 2-input ops require equal operand bases)
            qA = P.tile([128, N], BF16)   # q_h0 (rows 0:64) | q_h1 (64:128)
            kA = P.tile([128, N], BF16)   # k_h0 | k_h1
            qB = P.tile([64, N], BF16)    # q_h2
            kB = P.tile([64, N], BF16)    # k_h2
            # V token-major: [part=token%128, tile, head, 66] (64=V, col 64=1)
            vsb = P.tile([128, 16, NH, 66], BF16)
            vtu = [P.tile([64, max(u, 1)], BF16, name=f"vtu{_h}", tag=f"vtu{_h}")
                   for _h in range(NH)] if u else []
            aoA = P.tile([128, N], BF16)   # heads 0 (rows 0:64), 1 (64:128)
            aoB = P.tile([64, N], BF16)    # head 2
            onesf = P.tile([128, 80], F32)
            ones65 = P.tile([128, 65], BF16)  # diag-reduce lhsT (65 out rows)
            ones65b = ones65[64:128, :]

            # DMA order = first-exp critical path; descriptor gen serializes.
            nc.sync.dma_start(x16[:, :, 0:256], xd[:, :, 0:256])
            nc.sync.dma_start(wqk[:, 0, :, :], wqkd[:, 0, :, :])
            nc.sync.dma_start(wqk[:, 1:3, :, :], wqkd[:, 1:3, :, :])
            nc.sync.dma_start(x16[:, :, 256:512], xd[:, :, 256:512])
            nc.sync.dma_start(x16[:, :, 512:1024], xd[:, :, 512:1024])
            nc.sync.dma_start(wv[:], wvd[:])
            nc.sync.dma_start(x16[:, :, 1024:1536], xd[:, :, 1024:1536])
            nc.sync.dma_start(x16[:, :, 1536:2048], xd[:, :, 1536:2048])
            nc.sync.dma_start(wpA[:], wpAd[:])
            nc.sync.dma_start(wpB[:], wpBd[:])

            nc.vector.memset(onesf[:], 1.0)
            nc.vector.tensor_copy(ones65[:], onesf[:, 0:65])
            nc.vector.tensor_copy(
                vsb[:, :, :, 64:65],
                onesf[:, 0:16 * NH].rearrange("p (a b x) -> p a b x", b=NH,
                                              x=1))

            # head -> (q tile, k tile, row base)
            heads = [(qA, kA, 0), (qA, kA, 64), (qB, kB, 0)]

            with tc.tile_pool(name="ps", bufs=1, space="PSUM") as PS, \
                 tc.tile_pool(name="a16p", bufs=3) as ap, \
                 tc.tile_pool(name="scr", bufs=3) as sc, \
                 tc.tile_pool(name="ot", bufs=4) as ot:

                def psA(name):   # [128, 1024] f32 — S^T tiles (2 banks x 2)
                    return PS.tile([128, 1024], F32, name=name, tag="A", bufs=2)

                def psB(name):   # [128, 512] f32 — qkv/v/vtu/proj (1 bank x 2)
                    return PS.tile([128, 512], F32, name=name, tag="B", bufs=2)

                def psC(name):   # [65, 512] f32 — A@V accum + diag (1 bank x 2)
                    return PS.tile([65, 512], F32, name=name, tag="C", bufs=2)

                # warm-up: the PE clock ramps after ~3us of continuous
                # activity (HAM); run throwaway matmuls while the input DMA
                # is still in flight so the real QKV work starts at full rate
                wps = PS.tile([128, 512], F32, name="warm", tag="B", bufs=2)
                for _w in range(34):
                    nc.tensor.matmul(wps[0:65, 0:64], ones65[0:64, :],
                                     ones65[0:64, 0:64],
                                     start=True, stop=True,
                                     skip_group_check=True)

                # ---- phase 1: projections ----
                def emit_qk(c0, c1, early=False):
                    sl = slice(c0, c1)
                    w = c1 - c0
                    # 3 psum groups; the third holds q_h2|k_h2 stacked and
                    # is split into base-0 tiles by two copies (cross-
                    # partition copy *inputs* are HW-legal for one operand)
                    for gi in range(3):
                        ps = psB("qkps")
                        for t in range(6):
                            nc.tensor.matmul(
                                ps[:, 0:w], wqk[:, gi, t, :], x16[:, t, sl],
                                start=(t == 0), stop=(t == 5))
                        if gi < 2:
                            dst = (qA, kA)[gi]
                            if early and gi == 1:
                                # ACT is idle before the first exp
                                nc.scalar.copy(dst[:, sl], ps[:, 0:w])
                            else:
                                nc.vector.tensor_copy(dst[:, sl], ps[:, 0:w])
                        else:
                            nc.vector.tensor_copy(qB[:, sl], ps[0:64, 0:w])
                            eng = nc.scalar if early else nc.vector
                            eng_copy = (eng.copy if early
                                        else nc.vector.tensor_copy)
                            eng_copy(kB[:, sl], ps[64:128, 0:w])

                def emit_v(ch):
                    # V (token-major) for the visible token tiles of a chunk
                    nt0, nt1 = ch * 4, min((ch + 1) * 4, T)
                    for nt in range(nt0, nt1):
                        ps = psB("vps")
                        tsl = slice(nt * 128, nt * 128 + 128)
                        for t in range(6):
                            nc.tensor.matmul(
                                ps[:, 0:192], x16[:, t, tsl], wv[:, t, :],
                                start=(t == 0), stop=(t == 5))
                        nc.vector.tensor_copy(
                            vsb[:, nt, :, 0:64],
                            ps[:, 0:192].rearrange("p (h x) -> p h x", x=64))

                def emit_vtu():
                    # unseen V, feature-major per head (for the diag fixup)
                    for h in range(NH):
                        for uc in range(0, u, 512):
                            w = min(512, u - uc)
                            ps = psB("vtups")
                            for t in range(6):
                                nc.tensor.matmul(
                                    ps[0:64, 0:w],
                                    wv[:, t, h * 64:(h + 1) * 64],
                                    x16[:, t, kfull + uc:kfull + uc + w],
                                    start=(t == 0), stop=(t == 5))
                            nc.vector.tensor_copy(vtu[h][:, uc:uc + w],
                                                  ps[0:64, 0:w])

                def emit_st(Q, h, p):
                    qt, kt, bh = heads[h]
                    full = (2 * p + 2 <= T)
                    nw = 2 if full else 1
                    st = psA("stps")
                    a16 = ap.tile([128, 1024], BF16, tag="a16", name="a16")
                    for i in range(nw):
                        ksl = slice((2 * p + i) * 128, (2 * p + i) * 128 + 128)
                        nc.tensor.matmul(
                            st[:, i * 512:(i + 1) * 512], kt[bh:bh + 64, ksl],
                            qt[bh:bh + 64, Q * 512:Q * 512 + 512],
                            start=True, stop=True)
                    if 2 * p + nw == T and rem < 128:
                        # mask scores of keys >= kfull in the last tile
                        nc.vector.memset(
                            st[rem:128, (nw - 1) * 512:nw * 512], -1e30)
                    return st, a16, nw

                def emit_proj(Q, co, tail_dma=False):
                    qs = Q * 512
                    pj = psB("pjps")
                    nc.tensor.matmul(pj[:], wpA[:, co * 128:(co + 1) * 128],
                                     aoA[:, qs:qs + 512],
                                     start=True, stop=False)
                    nc.tensor.matmul(pj[:], wpB[:, co * 128:(co + 1) * 128],
                                     aoB[:, qs:qs + 512],
                                     start=False, stop=True)
                    o = ot.tile([128, 512], BF16, tag="o", name="o")
                    if tail_dma and co % 2 == 0:
                        nc.scalar.copy(o[:], pj[:])
                    else:
                        nc.vector.tensor_copy(o[:], pj[:])
                    nc.sync.dma_start(yd[co * 128:(co + 1) * 128, qs:qs + 512],
                                      o[:])

                # ---- phase 2+3: pipelined attention + projection ----
                # two (Q, h) blocks run interleaved: while one stream's A@V
                # waits on its exp, the PE executes the other stream's S^T
                # (the in-order PE queue would otherwise idle ~200ns/pair)
                blocks = [(Q, h) for Q in qorder for h in range(NH)]
                tasks = []
                for b0 in range(0, len(blocks), 2):
                    grp = blocks[b0:b0 + 2]
                    for p in range(PAIRS):
                        for Q, h in grp:
                            tasks.append((Q, h, p))
                sts = {}
                # chunk 0 first (two 256-col halves: shortest path to the
                # first S^T pairs), with the leading S^T work interleaved
                emit_qk(0, 256, early=True)
                emit_qk(256, 512, early=True)
                sts[tasks[0]] = emit_st(*tasks[0])
                sts[tasks[1]] = emit_st(*tasks[1])
                emit_qk(512, 1024)
                emit_v(0)
                emit_qk(1024, 1536)
                emit_v(1)
                emit_v(2)

                proj_q = []      # projection tiles ready to interleave
                deferred = None
                esbs = {}
                avs = {}
                for idx, (Q, h, p) in enumerate(tasks):
                    qs, qe = Q * 512, Q * 512 + 512
                    us = max(qs, kfull)
                    qt, kt, bh = heads[h]
                    if p == 0:
                        if us < qe and u:
                            # diagonal scores of the unseen queries
                            off = us - qs
                            prod = sc.tile([128, 512], BF16, tag="prod",
                                           name="prod")
                            nc.vector.tensor_mul(prod[bh:bh + 64, off:512],
                                                 qt[bh:bh + 64, us:qe],
                                                 kt[bh:bh + 64, us:qe])
                            dg = psB("dgps")
                            nc.tensor.matmul(dg[0:65, off:512],
                                             ones65[bh:bh + 64, :],
                                             prod[bh:bh + 64, off:512],
                                             start=True, stop=True)
                            esb = sc.tile([65, 512], BF16, tag="esb",
                                          name="esb")
                            nc.scalar.activation(esb[:, off:512],
                                                 dg[0:65, off:512], EXP,
                                                 scale=0.125)
                            esbs[(Q, h)] = esb
                        avs[(Q, h)] = psC("avps")
                    # stay one S^T pair ahead of the exp stream
                    if idx + 1 < len(tasks) and tasks[idx + 1] not in sts:
                        sts[tasks[idx + 1]] = emit_st(*tasks[idx + 1])
                    if idx == 4:
                        emit_qk(1536, 2048)
                        emit_vtu()
                    st, a16, nw = sts.pop((Q, h, p))
                    nc.scalar.activation(
                        a16[:, 0:nw * 512], st[:, 0:nw * 512], EXP,
                        scale=0.125)
                    if proj_q and (idx % 3 == 0 or len(proj_q) > 6):
                        # independent PE work placed before the exp-gated
                        # A@V matmuls; rate-limited so the S^T stream is
                        # never crowded out, but fast enough to drain all
                        # non-final chunks before the tail
                        emit_proj(*proj_q.pop(0))
                    av = avs[(Q, h)]
                    for i in range(nw):
                        nc.tensor.matmul(
                            av[:], vsb[:, 2 * p + i, h, 0:65],
                            a16[:, i * 512:(i + 1) * 512],
                            start=(p == 0 and i == 0),
                            stop=(p == PAIRS - 1 and i == nw - 1),
                            skip_group_check=True)
                    if p < PAIRS - 1:
                        continue
                    avs.pop((Q, h))
                    # ---- softmax normalization: av row 64 is the sum L ----
                    # rc at partition 0: partition_broadcast ignores AP
                    # partition offsets on HW
                    rc = sc.tile([1, 512], F32, tag="rc", name="rc")
                    blc = sc.tile([64, 512], F32, tag="blc", name="blc")
                    dsts = ((aoA, 0), (aoA, 64), (aoB, 0))
                    dt_, db = dsts[h]
                    if us < qe and u:
                        off = us - qs
                        esb = esbs.pop((Q, h))
                        lnew = sc.tile([1, 512], F32, tag="lnew", name="lnew")
                        if off:
                            nc.vector.tensor_copy(lnew[0:1, 0:off],
                                                  av[64:65, 0:off])
                        nc.vector.tensor_add(lnew[0:1, off:512],
                                             av[64:65, off:512],
                                             esb[64:65, off:512])
                        nc.vector.reciprocal(rc[0:1, :], lnew[0:1, :])
                        nc.gpsimd.partition_broadcast(blc[:], rc[0:1, :])
                        if off:
                            nc.vector.tensor_mul(dt_[db:db + 64, qs:us],
                                                 av[0:64, 0:off],
                                                 blc[:, 0:off])
                        t1 = sc.tile([64, 512], BF16, tag="t1", name="t1")
                        t2 = sc.tile([64, 512], F32, tag="t2", name="t2")
                        nc.vector.tensor_mul(t1[:, off:512],
                                             vtu[h][:, us - kfull:qe - kfull],
                                             esb[0:64, off:512])
                        nc.vector.tensor_add(t2[:, off:512],
                                             av[0:64, off:512], t1[:, off:512])
                        nc.vector.tensor_mul(dt_[db:db + 64, us:qe],
                                             t2[:, off:512], blc[:, off:512])
                    elif idx == len(tasks) - 2:
                        # tail: defer this stream's mul behind the final
                        # stream's reciprocal so the two DVE chains overlap
                        # the Pool broadcasts instead of serializing
                        nc.vector.reciprocal(rc[0:1, :], av[64:65, :])
                        nc.gpsimd.partition_broadcast(blc[:], rc[0:1, :])
                        deferred = (av, blc, dt_, db, qs, qe)
                    elif idx == len(tasks) - 1:
                        nc.vector.reciprocal(rc[0:1, :], av[64:65, :])
                        nc.gpsimd.partition_broadcast(blc[:], rc[0:1, :])
                        if deferred is not None:
                            av2, blc2, dt2, db2, qs2, qe2 = deferred
                            nc.vector.tensor_mul(dt2[db2:db2 + 64, qs2:qe2],
                                                 av2[0:64, :], blc2[:])
                        nc.vector.tensor_mul(dt_[db:db + 64, qs:qe],
                                             av[0:64, :], blc[:])
                    else:
                        nc.vector.reciprocal(rc[0:1, :], av[64:65, :])
                        nc.gpsimd.partition_broadcast(blc[:], rc[0:1, :])
                        nc.vector.tensor_mul(dt_[db:db + 64, qs:qe],
                                             av[0:64, :], blc[:])
                    if h == NH - 1:
                        proj_q.extend((Q, co) for co in range(6))
                # drain any straggler tiles of non-final chunks first
                while len(proj_q) > 6:
                    emit_proj(*proj_q.pop(0))
                assert len({q for q, _ in proj_q}) <= 1
                # final chunk's projections: co-pairs share one A-tag
                # psum tile -> one copy + one DMA per pair (fewer serialized
                # descriptor-gens on the tail critical path)
                qs = proj_q[0][0] * 512 if proj_q else 0
                for cp in range(3):
                    pj = psA("pjtail")
                    # wpA half first for both blocks: it only needs aoA
                    # (ready one norm earlier than aoB), so the PE works
                    # through it while the final norm chain completes
                    for ci in range(2):
                        co = 2 * cp + ci
                        csl = slice(ci * 512, (ci + 1) * 512)
                        nc.tensor.matmul(pj[:, csl],
                                         wpA[:, co * 128:(co + 1) * 128],
                                         aoA[:, qs:qs + 512],
                                         start=True, stop=False,
                                         skip_group_check=True)
                    for ci in range(2):
                        co = 2 * cp + ci
                        csl = slice(ci * 512, (ci + 1) * 512)
                        nc.tensor.matmul(pj[:, csl],
                                         wpB[:, co * 128:(co + 1) * 128],
                                         aoB[:, qs:qs + 512],
                                         start=False, stop=True,
                                         skip_group_check=True)
                    o = ot.tile([128, 1024], BF16, tag="o2", name="o2", bufs=3)
                    nc.scalar.copy(o[:, 0:512], pj[:, 0:512])
                    nc.vector.tensor_copy(o[:, 512:1024], pj[:, 512:1024])
                    nc.sync.dma_start(
                        yd[cp * 256:(cp + 1) * 256, qs:qs + 512].rearrange(
                            "(i p) c -> p i c", i=2),
                        o[:].rearrange("p (i c) -> p i c", i=2))

    nc.compile()
    return nc


def _fold(m):
    """[768, F] -> [128, 6, F] bf16: partition p of k-tile t = feature
    128*t + p."""
    F = m.shape[1]
    return np.ascontiguousarray(
        m.reshape(6, 128, F).transpose(1, 0, 2)).astype(ml_dtypes.bfloat16)


def kernel(**inputs):
    global _last_results
    from concourse.bass_utils import run_bass_kernel_spmd

    x = np.asarray(inputs["x"], np.float32)
    w_qkv = np.asarray(inputs["w_qkv"], np.float32)
    w_proj = np.asarray(inputs["w_proj"], np.float32)
    b_proj = np.asarray(inputs["b_proj"], np.float32)
    u = int(np.asarray(inputs["unseen_size"]))
    B = x.shape[0]

    nc = _build(u)

    wT = np.ascontiguousarray(w_qkv.T)         # [768, 2304]
    wpT_full = np.ascontiguousarray(w_proj.T)  # [768, 768] (ci, co)
    x16b = [_fold(np.ascontiguousarray(x[b].T)) for b in range(B)]

    in_maps = []
    for core in range(8):
        b, g = divmod(core, 4)
        hs = [3 * g, 3 * g + 1, 3 * g + 2]
        qcols = [h * D + i for h in hs[:2] for i in range(D)]
        kcols = [C + h * D + i for h in hs[:2] for i in range(D)]
        q2 = [hs[2] * D + i for i in range(D)]
        k2 = [C + hs[2] * D + i for i in range(D)]
        vcols = [2 * C + h * D + i for h in hs for i in range(D)]
        wqk16 = np.ascontiguousarray(
            _fold(wT[:, qcols + kcols + q2 + k2])
            .reshape(128, 6, 3, 128).transpose(0, 2, 1, 3))
        wv16 = _fold(wT[:, vcols])
        ci = [h * D + i for h in hs for i in range(D)]
        wpT = np.ascontiguousarray(wpT_full[ci, :])
        in_maps.append({
            "x16": x16b[b], "wqk16": wqk16, "wv16": wv16,
            "wpA": wpT[0:128].astype(ml_dtypes.bfloat16),
            "wpB": wpT[128:192].astype(ml_dtypes.bfloat16),
        })

    trace = bool(int(os.environ.get("KERNEL_TRACE", "0")))
    res = run_bass_kernel_spmd(nc, in_maps, core_ids=list(range(8)), trace=trace)
    _last_results = res

    y = np.zeros((B, N, C), np.float32)
    for core in range(8):
        b = core // 4
        y[b] += np.asarray(res.results[core]["yT16"]).astype(np.float32).T
    y += b_proj
    return y


# revision 64
# speedup vs baseline: 1.0088x; 1.0071x over previous
"""MCCDecoderAttention Trainium2 kernel (8 NeuronCores) — v3 (all-bf16).

Sharding: core = b*4 + g  (b in {0,1} batch, g in {0..3} head-group).
Each core computes attention for 3 heads of one batch plus its partial
contribution to the output projection; the host sums the 4 partials per
batch and adds b_proj.

Numerics: everything is bf16 (fp8 fails here: max logits reach ~9.5 so
exp overflows fp8e4m3, and the softmax is concentrated enough that fp8
quantization of A or V alone costs 3-4e-2 relative error).  Measured
end-to-end error of this scheme is ~7e-3 against the f32 reference.

Schedule (learned from TimelineSim iteration):
  * The PE executes its queue in order, so emission order is the
    schedule: S^T matmuls run one pair ahead of the exp stream, each
    finished chunk's projection is interleaved into the next chunk's
    pair stream, and chunk-3 QKV work is injected after the stream
    starts (its DMA lands last).
  * exp runs on ACT; PSUM-touching copies on DVE (GPSIMD cannot access
    PSUM); 1/L row broadcast via gpsimd partition_broadcast (source must
    sit at partition 0 of its tile - the HW ignores AP offsets there).
  * Output projection contracts K=192 as 128+64 via a stacked ao tile
    (cross-partition elementwise *output* is HW-legal; cross-partition
    inputs are not, hence the aligned-copy in the diag path).
  * Throwaway warm-up matmuls run while the input DMA is in flight so
    the PE clock (HAM) is at full rate when the real QKV work starts.

Decoder mask (last `u` keys masked except the diagonal) is handled by
looping keys over [0, N-u) plus an elementwise diagonal correction for
queries in the unseen range (per-head V^T of the unseen tokens: vtu).
"""

import functools
import os
import sys

for _p in ("/opt/trn_rl_repo", "/root/.axon_site/_ro/trn_rl_repo"):
    if os.path.isdir(_p) and _p not in sys.path:
        sys.path.insert(0, _p)

import numpy as np
import ml_dtypes

import concourse.bacc as bacc
import concourse.tile as tile
from concourse import mybir

N, C, D = 2048, 768, 64
NH = 3            # heads per core
F32 = mybir.dt.float32
BF16 = mybir.dt.bfloat16
EXP = mybir.ActivationFunctionType.Exp

_last_results = None  # BassKernelResults of the most recent run (for test.py)


@functools.lru_cache(maxsize=4)
def _build(u: int):
    nc = bacc.Bacc(None, target_bir_lowering=False)
    xd = nc.dram_tensor("x16", [128, 6, N], BF16, kind="ExternalInput")
    wqkd = nc.dram_tensor("wqk16", [128, 3, 6, 128], BF16,
                          kind="ExternalInput")
    wvd = nc.dram_tensor("wv16", [128, 6, 192], BF16, kind="ExternalInput")
    wpAd = nc.dram_tensor("wpA", [128, C], BF16, kind="ExternalInput")
    wpBd = nc.dram_tensor("wpB", [64, C], BF16, kind="ExternalInput")
    yd = nc.dram_tensor("yT16", [C, N], BF16, kind="ExternalOutput")

    kfull = N - u
    T = (kfull + 127) // 128       # 128-key tiles covering the visible keys
    PAIRS = (T + 1) // 2           # pairs of key tiles (one st/exp per pair)
    rem = kfull - (T - 1) * 128    # valid keys in the last 128-tile (1..128)
    NQ = N // 512                  # query chunks
    # diag-corrected chunk early, cheap chunk last (short tail)
    qorder = [0] + list(range(NQ - 1, 0, -1)) if u else list(range(NQ))

    with nc.allow_low_precision(reason="bf16 staging"), \
         tile.TileContext(nc) as tc:
        with tc.tile_pool(name="persist", bufs=1) as P:
            x16 = P.tile([128, 6, N], BF16)
            wqk = P.tile([128, 3, 6, 128], BF16)
            wv = P.tile([128, 6, 192], BF16)
            wpA = P.tile([128, C], BF16)
            wpB = P.tile([64, C], BF16)
            # q/k tiles; per-head q,k share a partition base (matmul and
            # DVE 2-input ops require equal operand bases)
            qA = P.tile([128, N], BF16)   # q_h0 (rows 0:64) | q_h1 (64:128)
            kA = P.tile([128, N], BF16)   # k_h0 | k_h1
            qB = P.tile([64, N], BF16)    # q_h2
            kB = P.tile([64, N], BF16)    # k_h2
            # V token-major: [part=token%128, tile, head, 66] (64=V, col 64=1)
            vsb = P.tile([128, 16, NH, 66], BF16)
            vtu = [P.tile([64, max(u, 1)], BF16, name=f"vtu{_h}", tag=f"vtu{_h}")
                   for _h in range(NH)] if u else []
            aoA = P.tile([128, N], BF16)   # heads 0 (rows 0:64), 1 (64:128)
            aoB = P.tile([64, N], BF16)    # head 2
            onesf = P.tile([128, 80], F32)
            ones65 = P.tile([128, 65], BF16)  # diag-reduce lhsT (65 out rows)
            ones65b = ones65[64:128, :]

            # DMA order = first-exp critical path; descriptor gen serializes.
            nc.sync.dma_start(x16[:, :, 0:256], xd[:, :, 0:256])
            nc.sync.dma_start(wqk[:, 0, :, :], wqkd[:, 0, :, :])
            nc.sync.dma_start(wqk[:, 1:3, :, :], wqkd[:, 1:3, :, :])
            nc.sync.dma_start(x16[:, :, 256:512], xd[:, :, 256:512])
            nc.sync.dma_start(x16[:, :, 512:1024], xd[:, :, 512:1024])
            nc.sync.dma_start(wv[:], wvd[:])
            nc.sync.dma_start(x16[:, :, 1024:1536], xd[:, :, 1024:1536])
            nc.sync.dma_start(x16[:, :, 1536:2048], xd[:, :, 1536:2048])
            nc.sync.dma_start(wpA[:], wpAd[:])
            nc.sync.dma_start(wpB[:], wpBd[:])

            nc.vector.memset(onesf[:], 1.0)
            nc.vector.tensor_copy(ones65[:], onesf[:, 0:65])
            nc.vector.tensor_copy(
                vsb[:, :, :, 64:65],
                onesf[:, 0:16 * NH].rearrange("p (a b x) -> p a b x", b=NH,
                                              x=1))

            # head -> (q tile, k tile, row base)
            heads = [(qA, kA, 0), (qA, kA, 64), (qB, kB, 0)]

            with tc.tile_pool(name="ps", bufs=1, space="PSUM") as PS, \
                 tc.tile_pool(name="a16p", bufs=3) as ap, \
                 tc.tile_pool(name="scr", bufs=3) as sc, \
                 tc.tile_pool(name="ot", bufs=4) as ot:

                def psA(name):   # [128, 1024] f32 — S^T tiles (2 banks x 2)
                    return PS.tile([128, 1024], F32, name=name, tag="A", bufs=2)

                def psB(name):   # [128, 512] f32 — qkv/v/vtu/proj (1 bank x 2)
                    return PS.tile([128, 512], F32, name=name, tag="B", bufs=2)

                def psC(name):   # [65, 512] f32 — A@V accum + diag (1 bank x 2)
                    return PS.tile([65, 512], F32, name=name, tag="C", bufs=2)

                # warm-up: the PE clock ramps after ~3us of continuous
                # activity (HAM); run throwaway matmuls while the input DMA
                # is still in flight so the real QKV work starts at full rate
                wps = PS.tile([128, 512], F32, name="warm", tag="B", bufs=2)
                for _w in range(34):
                    nc.tensor.matmul(wps[0:65, 0:64], ones65[0:64, :],
                                     ones65[0:64, 0:64],
                                     start=True, stop=True,
                                     skip_group_check=True)

                # ---- phase 1: projections ----
                def emit_qk(c0, c1, early=False):
                    sl = slice(c0, c1)
                    w = c1 - c0
                    # 3 psum groups; the third holds q_h2|k_h2 stacked and
                    # is split into base-0 tiles by two copies (cross-
                    # partition copy *inputs* are HW-legal for one operand)
                    for gi in range(3):
                        ps = psB("qkps")
                        for t in range(6):
                            nc.tensor.matmul(
                                ps[:, 0:w], wqk[:, gi, t, :], x16[:, t, sl],
                                start=(t == 0), stop=(t == 5))
                        if gi < 2:
                            dst = (qA, kA)[gi]
                            if early and gi == 1:
                                # ACT is idle before the first exp
                                nc.scalar.copy(dst[:, sl], ps[:, 0:w])
                            else:
                                nc.vector.tensor_copy(dst[:, sl], ps[:, 0:w])
                        else:
                            nc.vector.tensor_copy(qB[:, sl], ps[0:64, 0:w])
                            eng = nc.scalar if early else nc.vector
                            eng_copy = (eng.copy if early
                                        else nc.vector.tensor_copy)
                            eng_copy(kB[:, sl], ps[64:128, 0:w])

                def emit_v(ch):
                    # V (token-major) for the visible token tiles of a chunk
                    nt0, nt1 = ch * 4, min((ch + 1) * 4, T)
                    for nt in range(nt0, nt1):
                        ps = psB("vps")
                        tsl = slice(nt * 128, nt * 128 + 128)
                        for t in range(6):
                            nc.tensor.matmul(
                                ps[:, 0:192], x16[:, t, tsl], wv[:, t, :],
                                start=(t == 0), stop=(t == 5))
                        nc.vector.tensor_copy(
                            vsb[:, nt, :, 0:64],
                            ps[:, 0:192].rearrange("p (h x) -> p h x", x=64))

                def emit_vtu():
                    # unseen V, feature-major per head (for the diag fixup)
                    for h in range(NH):
                        for uc in range(0, u, 512):
                            w = min(512, u - uc)
                            ps = psB("vtups")
                            for t in range(6):
                                nc.tensor.matmul(
                                    ps[0:64, 0:w],
                                    wv[:, t, h * 64:(h + 1) * 64],
                                    x16[:, t, kfull + uc:kfull + uc + w],
                                    start=(t == 0), stop=(t == 5))
                            nc.vector.tensor_copy(vtu[h][:, uc:uc + w],
                                                  ps[0:64, 0:w])

                def emit_st(Q, h, p):
                    qt, kt, bh = heads[h]
                    full = (2 * p + 2 <= T)
                    nw = 2 if full else 1
                    st = psA("stps")
                    a16 = ap.tile([128, 1024], BF16, tag="a16", name="a16")
                    for i in range(nw):
                        ksl = slice((2 * p + i) * 128, (2 * p + i) * 128 + 128)
                        nc.tensor.matmul(
                            st[:, i * 512:(i + 1) * 512], kt[bh:bh + 64, ksl],
                            qt[bh:bh + 64, Q * 512:Q * 512 + 512],
                            start=True, stop=True)
                    if 2 * p + nw == T and rem < 128:
                        # mask scores of keys >= kfull in the last tile
                        nc.vector.memset(
                            st[rem:128, (nw - 1) * 512:nw * 512], -1e30)
                    return st, a16, nw

                def emit_proj(Q, co, tail_dma=False):
                    qs = Q * 512
                    pj = psB("pjps")
                    nc.tensor.matmul(pj[:], wpA[:, co * 128:(co + 1) * 128],
                                     aoA[:, qs:qs + 512],
                                     start=True, stop=False)
                    nc.tensor.matmul(pj[:], wpB[:, co * 128:(co + 1) * 128],
                                     aoB[:, qs:qs + 512],
                                     start=False, stop=True)
                    o = ot.tile([128, 512], BF16, tag="o", name="o")
                    if tail_dma and co % 2 == 0:
                        nc.scalar.copy(o[:], pj[:])
                    else:
                        nc.vector.tensor_copy(o[:], pj[:])
                    nc.sync.dma_start(yd[co * 128:(co + 1) * 128, qs:qs + 512],
                                      o[:])

                # ---- phase 2+3: pipelined attention + projection ----
                # two (Q, h) blocks run interleaved: while one stream's A@V
                # waits on its exp, the PE executes the other stream's S^T
                # (the in-order PE queue would otherwise idle ~200ns/pair)
                blocks = [(Q, h) for Q in qorder for h in range(NH)]
                tasks = []
                for b0 in range(0, len(blocks), 2):
                    grp = blocks[b0:b0 + 2]
                    for p in range(PAIRS):
                        for Q, h in grp:
                            tasks.append((Q, h, p))
                sts = {}
                # chunk 0 first (two 256-col halves: shortest path to the
                # first S^T pairs), with the leading S^T work interleaved
                emit_qk(0, 256, early=True)
                emit_qk(256, 512, early=True)
                sts[tasks[0]] = emit_st(*tasks[0])
                sts[tasks[1]] = emit_st(*tasks[1])
                emit_qk(512, 1024)
                emit_v(0)
                emit_qk(1024, 1536)
                emit_v(1)
                emit_v(2)

                proj_q = []      # projection tiles ready to interleave
                deferred = None
                esbs = {}
                avs = {}
                for idx, (Q, h, p) in enumerate(tasks):
                    qs, qe = Q * 512, Q * 512 + 512
                    us = max(qs, kfull)
                    qt, kt, bh = heads[h]
                    if p == 0:
                        if us < qe and u:
                            # diagonal scores of the unseen queries
                            off = us - qs
                            prod = sc.tile([128, 512], BF16, tag="prod",
                                           name="prod")
                            nc.vector.tensor_mul(prod[bh:bh + 64, off:512],
                                                 qt[bh:bh + 64, us:qe],
                                                 kt[bh:bh + 64, us:qe])
                            dg = psB("dgps")
                            nc.tensor.matmul(dg[0:65, off:512],
                                             ones65[bh:bh + 64, :],
                                             prod[bh:bh + 64, off:512],
                                             start=True, stop=True)
                            esb = sc.tile([65, 512], BF16, tag="esb",
                                          name="esb")
                            nc.scalar.activation(esb[:, off:512],
                                                 dg[0:65, off:512], EXP,
                                                 scale=0.125)
                            esbs[(Q, h)] = esb
                        avs[(Q, h)] = psC("avps")
                    # stay one S^T pair ahead of the exp stream
                    if idx + 1 < len(tasks) and tasks[idx + 1] not in sts:
                        sts[tasks[idx + 1]] = emit_st(*tasks[idx + 1])
                    if idx == 4:
                        emit_qk(1536, 2048)
                        emit_vtu()
                    st, a16, nw = sts.pop((Q, h, p))
                    nc.scalar.activation(
                        a16[:, 0:nw * 512], st[:, 0:nw * 512], EXP,
                        scale=0.125)
                    if proj_q and (idx % 3 == 0 or len(proj_q) > 6):
                        # independent PE work placed before the exp-gated
                        # A@V matmuls; rate-limited so the S^T stream is
                        # never crowded out, but fast enough to drain all
                        # non-final chunks before the tail
                        emit_proj(*proj_q.pop(0))
                    av = avs[(Q, h)]
                    for i in range(nw):
                        nc.tensor.matmul(
                            av[:], vsb[:, 2 * p + i, h, 0:65],
                            a16[:, i * 512:(i + 1) * 512],
                            start=(p == 0 and i == 0),
                            stop=(p == PAIRS - 1 and i == nw - 1),
                            skip_group_check=True)
                    if p < PAIRS - 1:
                        continue
                    avs.pop((Q, h))
                    # ---- softmax normalization: av row 64 is the sum L ----
                    # rc at partition 0: partition_broadcast ignores AP
                    # partition offsets on HW
                    rc = sc.tile([1, 512], F32, tag="rc", name="rc")
                    blc = sc.tile([64, 512], F32, tag="blc", name="blc")
                    dsts = ((aoA, 0), (aoA, 64), (aoB, 0))
                    dt_, db = dsts[h]
                    if us < qe and u:
                        off = us - qs
                        esb = esbs.pop((Q, h))
                        lnew = sc.tile([1, 512], F32, tag="lnew", name="lnew")
                        if off:
                            nc.vector.tensor_copy(lnew[0:1, 0:off],
                                                  av[64:65, 0:off])
                        nc.vector.tensor_add(lnew[0:1, off:512],
                                             av[64:65, off:512],
                                             esb[64:65, off:512])
                        nc.vector.reciprocal(rc[0:1, :], lnew[0:1, :])
                        nc.gpsimd.partition_broadcast(blc[:], rc[0:1, :])
                        if off:
                            nc.vector.tensor_mul(dt_[db:db + 64, qs:us],
                                                 av[0:64, 0:off],
                                                 blc[:, 0:off])
                        t1 = sc.tile([64, 512], BF16, tag="t1", name="t1")
                        t2 = sc.tile([64, 512], F32, tag="t2", name="t2")
                        nc.vector.tensor_mul(t1[:, off:512],
                                             vtu[h][:, us - kfull:qe - kfull],
                                             esb[0:64, off:512])
                        nc.vector.tensor_add(t2[:, off:512],
                                             av[0:64, off:512], t1[:, off:512])
                        nc.vector.tensor_mul(dt_[db:db + 64, us:qe],
                                             t2[:, off:512], blc[:, off:512])
                    elif idx == len(tasks) - 2:
                        # tail: defer this stream's mul behind the final
                        # stream's reciprocal so the two DVE chains overlap
                        # the Pool broadcasts instead of serializing
                        nc.vector.reciprocal(rc[0:1, :], av[64:65, :])
                        nc.gpsimd.partition_broadcast(blc[:], rc[0:1, :])
                        deferred = (av, blc, dt_, db, qs, qe)
                    elif idx == len(tasks) - 1:
                        nc.vector.reciprocal(rc[0:1, :], av[64:65, :])
                        nc.gpsimd.partition_broadcast(blc[:], rc[0:1, :])
                        if deferred is not None:
                            av2, blc2, dt2, db2, qs2, qe2 = deferred
                            nc.vector.tensor_mul(dt2[db2:db2 + 64, qs2:qe2],
                                                 av2[0:64, :], blc2[:])
                        nc.vector.tensor_mul(dt_[db:db + 64, qs:qe],
                                             av[0:64, :], blc[:])
                    else:
                        nc.vector.reciprocal(rc[0:1, :], av[64:65, :])
                        nc.gpsimd.partition_broadcast(blc[:], rc[0:1, :])
                        nc.vector.tensor_mul(dt_[db:db + 64, qs:qe],
                                             av[0:64, :], blc[:])
                    if h == NH - 1:
                        proj_q.extend((Q, co) for co in range(6))
                # drain any straggler tiles of non-final chunks first
                while len(proj_q) > 6:
                    emit_proj(*proj_q.pop(0))
                assert len({q for q, _ in proj_q}) <= 1
                # final chunk's projections: co-pairs share one A-tag
                # psum tile -> one copy + one DMA per pair (fewer serialized
                # descriptor-gens on the tail critical path)
                qs = proj_q[0][0] * 512 if proj_q else 0
                for cp in range(2):
                    pj = psA("pjtail")
                    # wpA half first for both blocks: it only needs aoA
                    # (ready one norm earlier than aoB), so the PE works
                    # through it while the final norm chain completes
                    for ci in range(2):
                        co = 2 * cp + ci
                        csl = slice(ci * 512, (ci + 1) * 512)
                        nc.tensor.matmul(pj[:, csl],
                                         wpA[:, co * 128:(co + 1) * 128],
                                         aoA[:, qs:qs + 512],
                                         start=True, stop=False,
                                         skip_group_check=True)
                    for ci in range(2):
                        co = 2 * cp + ci
                        csl = slice(ci * 512, (ci + 1) * 512)
                        nc.tensor.matmul(pj[:, csl],
                                         wpB[:, co * 128:(co + 1) * 128],
                                         aoB[:, qs:qs + 512],
                                         start=False, stop=True,
                                         skip_group_check=True)
                    o = ot.tile([128, 1024], BF16, tag="o2", name="o2", bufs=3)
                    nc.scalar.copy(o[:, 0:512], pj[:, 0:512])
                    nc.vector.tensor_copy(o[:, 512:1024], pj[:, 512:1024])
                    nc.sync.dma_start(
                        yd[cp * 256:(cp + 1) * 256, qs:qs + 512].rearrange(
                            "(i p) c -> p i c", i=2),
                        o[:].rearrange("p (i c) -> p i c", i=2))
                if proj_q:
                    # last two blocks on B-tag tiles: no A-tag alloc wait,
                    # and the final DMA transfer is half the size
                    emit_proj(proj_q[0][0], 4, tail_dma=True)
                    emit_proj(proj_q[0][0], 5, tail_dma=False)

    nc.compile()
    return nc


def _fold(m):
    """[768, F] -> [128, 6, F] bf16: partition p of k-tile t = feature
    128*t + p."""
    F = m.shape[1]
    return np.ascontiguousarray(
        m.reshape(6, 128, F).transpose(1, 0, 2)).astype(ml_dtypes.bfloat16)


def kernel(**inputs):
    global _last_results
    from concourse.bass_utils import run_bass_kernel_spmd

    x = np.asarray(inputs["x"], np.float32)
    w_qkv = np.asarray(inputs["w_qkv"], np.float32)
    w_proj = np.asarray(inputs["w_proj"], np.float32)
    b_proj = np.asarray(inputs["b_proj"], np.float32)
    u = int(np.asarray(inputs["unseen_size"]))
    B = x.shape[0]

    nc = _build(u)

    wT = np.ascontiguousarray(w_qkv.T)         # [768, 2304]
    wpT_full = np.ascontiguousarray(w_proj.T)  # [768, 768] (ci, co)
    x16b = [_fold(np.ascontiguousarray(x[b].T)) for b in range(B)]

    in_maps = []
    for core in range(8):
        b, g = divmod(core, 4)
        hs = [3 * g, 3 * g + 1, 3 * g + 2]
        qcols = [h * D + i for h in hs[:2] for i in range(D)]
        kcols = [C + h * D + i for h in hs[:2] for i in range(D)]
        q2 = [hs[2] * D + i for i in range(D)]
        k2 = [C + hs[2] * D + i for i in range(D)]
        vcols = [2 * C + h * D + i for h in hs for i in range(D)]
        wqk16 = np.ascontiguousarray(
            _fold(wT[:, qcols + kcols + q2 + k2])
            .reshape(128, 6, 3, 128).transpose(0, 2, 1, 3))
        wv16 = _fold(wT[:, vcols])
        ci = [h * D + i for h in hs for i in range(D)]
        wpT = np.ascontiguousarray(wpT_full[ci, :])
        in_maps.append({
            "x16": x16b[b], "wqk16": wqk16, "wv16": wv16,
            "wpA": wpT[0:128].astype(ml_dtypes.bfloat16),
            "wpB": wpT[128:192].astype(ml_dtypes.bfloat16),
        })

    trace = bool(int(os.environ.get("KERNEL_TRACE", "0")))
    res = run_bass_kernel_spmd(nc, in_maps, core_ids=list(range(8)), trace=trace)
    _last_results = res

    y = np.zeros((B, N, C), np.float32)
    for core in range(8):
        b = core // 4
        y[b] += np.asarray(res.results[core]["yT16"]).astype(np.float32).T
    y += b_proj
    return y
